# revision 16
# baseline (speedup 1.0000x reference)
"""Trainium2 Bass kernel for nn_ActivityAugmentation.

Pipeline (per batch sample b, time t, channel c):
  1. jitter:   xj = x + noise * 0.01
  2. scale:    * (0.9 + scale_u * 0.2)            [folded into warp weights]
  3. timewarp: y[t] = xj[i0[t]] * w0[t] + xj[i0[t]+1] * w1[t]
  4. rotation of channels 0,1 by per-sample angle  [commutes with 3, done pre-warp]
  5. channel dropout mask                          [zeroing, done pre-warp]

Sharding: pure data-parallel over batch, 64 samples per NeuronCore (8 cores).

Device strategy per core:
  - batch is processed in 8 groups of 8 samples. Each group's x/noise are
    loaded as SBUF slabs laid out [s_local(128 partitions), s_block(16),
    b(8), c(64)].
  - jitter via one scalar_tensor_tensor pass; rotation via 6 strided
    tensor_tensor ops on the c0/c1 columns; dropout via memset on dropped
    channel columns.
  - the time warp (a 2-tap gather/interp along T with indices shared by all
    (b, c)) is expressed as a sparse banded matrix W (2 nonzeros per row,
    scale folded in) and executed on TensorE as float32r matmuls:
      out[tb] (128 x 512) = sum_sb  Wblk[sb,tb].T @ slab[sb]   (PSUM accum)
  - PSUM evicted by ScalarE copies, stored back with per-(group, t-block)
    DMAs.
"""

import os
import numpy as np

import concourse.bacc as bacc
import concourse.mybir as mybir
from concourse.tile import TileContext
from concourse.bass_utils import run_bass_kernel_spmd

B, T, C = 512, 2048, 64
JITTER_STD = 0.01
SCALE_LO, SCALE_HI = 0.9, 1.1
TW_SIGMA = 0.2

N_CORES = 8
BS = B // N_CORES  # 64 batch samples per core
GB = 8             # batch samples per group (free dim = GB*C = 512)
NG = BS // GB      # 8 groups
P = 128
NTB = T // P       # 16 t-blocks
F = GB * C         # 512

F32 = mybir.dt.float32
F32R = mybir.dt.float32r
BF16 = mybir.dt.bfloat16


def _warp_params(warp_noise):
    """Replicate the reference's fp32 warp math on host (cheap, O(T))."""
    wn = np.asarray(warp_noise, dtype=np.float32)
    warp = np.cumsum(wn * np.float32(TW_SIGMA / T), dtype=np.float32)
    warp = (warp - warp[0]).astype(np.float32)
    warp = (warp / (warp[-1] + np.float32(1e-8))).astype(np.float32)
    t_orig = np.linspace(0.0, 1.0, T, dtype=np.float32)
    t_warped = np.clip(t_orig + warp * np.float32(0.2), np.float32(0.0), np.float32(1.0)).astype(np.float32)
    pos = (t_warped * np.float32(T - 1)).astype(np.float32)
    i0 = np.clip(np.floor(pos).astype(np.int32), 0, T - 2)
    frac = (pos - i0.astype(np.float32)).astype(np.float32)
    return i0, frac


def _build_w_blocks(i0, frac, scale):
    """Sparse banded warp matrix as a list of 128x128 lhsT blocks + schedule.

    Returns (wmat [NBLK,128,128] fp32 with lhsT layout [s_local, t_local],
             sched: list over t-blocks of lists of (s_block, blk_idx)).
    """
    w0 = (scale * (np.float32(1.0) - frac)).astype(np.float32)
    w1 = (scale * frac).astype(np.float32)
    blocks = []
    sched = []
    for tb in range(NTB):
        tl = np.arange(tb * P, (tb + 1) * P)
        per_sb = {}
        for idx, wgt in ((i0[tl], w0[tl]), (i0[tl] + 1, w1[tl])):
            nz = wgt != 0.0
            for sb in np.unique(idx[nz] // P):
                m = nz & (idx // P == sb)
                blk = per_sb.setdefault(int(sb), np.zeros((P, P), np.float32))
                np.add.at(blk, (idx[m] - sb * P, np.arange(P)[m]), wgt[m])
        entry = []
        for sb in sorted(per_sb):
            entry.append((sb, len(blocks)))
            blocks.append(per_sb[sb])
        sched.append(entry)
    wmat = np.stack(blocks).astype(np.float32)
    return wmat, sched


def _build_nc(nblk, sched, dropped, rot_needed, iters=1):
    dma_only = bool(int(os.environ.get("KERNEL_DMA_ONLY", "0")))
    store4 = bool(int(os.environ.get("KERNEL_STORE4", "1")))
    skip = set(os.environ.get("KERNEL_SKIP", "").split(","))
    jsplit = bool(int(os.environ.get("KERNEL_JSPLIT", "0")))
    nc = bacc.Bacc(trn_type="TRN2")
    xin = nc.declare_dram_parameter("x", [BS, T, C], F32R, isOutput=False)
    nin = nc.declare_dram_parameter("n", [BS, T, C], BF16, isOutput=False)
    win = nc.declare_dram_parameter("w", [nblk, P, P], F32R, isOutput=False)
    rin = nc.declare_dram_parameter("rot", [2 * NG, P, P], F32, isOutput=False)
    # device-friendly store layout: per (g, tb) a (P, GB*C) block whose rows
    # are 2KB-contiguous in DRAM (vs 256B runs for the natural (B,T,C) layout)
    out = nc.declare_dram_parameter("out", [NG, NTB, P, GB * C], F32, isOutput=True)

    with TileContext(nc) as tc:
        with (
            tc.tile_pool(name="consts", bufs=1) as cpool,
            tc.tile_pool(name="xs", bufs=int(os.environ.get("KERNEL_XBUFS", "2"))) as xpool,
            tc.tile_pool(name="ns", bufs=int(os.environ.get("KERNEL_NBUFS", "2"))) as npool,
            tc.tile_pool(name="tmp", bufs=2) as tpool,
            tc.tile_pool(name="ot", bufs=int(os.environ.get("KERNEL_OBUFS", "4"))) as opool,
            tc.tile_pool(name="psum", bufs=6, space="PSUM") as ppool,
        ):
            wt = cpool.tile([P, nblk, P], F32R)
            nc.sync.dma_start(out=wt[:], in_=win.rearrange("k s t -> s k t"))
            rt = cpool.tile([P, 2, NG, P], F32)
            nc.sync.dma_start(
                out=rt[:].rearrange("p a g q -> p (a g) q"),
                in_=rin.rearrange("k p q -> p k q"),
            )

            for g in range(NG * iters):
                g = g % NG
                xs = xpool.tile([P, NTB, GB, C], F32R)
                ns = npool.tile([P, NTB, GB, C], BF16)
                for b in range(GB):
                    nc.sync.dma_start(
                        out=xs[:, :, b, :],
                        in_=xin[g * GB + b].rearrange("(sb p) c -> p sb c", p=P),
                    )
                    nc.sync.dma_start(
                        out=ns[:, :, b, :],
                        in_=nin[g * GB + b].rearrange("(sb p) c -> p sb c", p=P),
                    )
                if dma_only:
                    for tb in range(NTB):
                        nc.scalar.dma_start(
                            out=out[g, tb],
                            in_=xs[:, tb, :, :].bitcast(F32),
                        )
                    continue
                # jitter: xs = noise*0.01 + xs  (in place)
                if "j" not in skip:
                    jeng = nc.gpsimd if (jsplit and g % 2 == 1) else nc.vector
                    jeng.scalar_tensor_tensor(
                        out=xs[:],
                        in0=ns[:],
                        scalar=JITTER_STD,
                        in1=xs[:],
                        op0=mybir.AluOpType.mult,
                        op1=mybir.AluOpType.add,
                    )
                # rotation of channels 0,1 (commutes with warp)
                if rot_needed and "r" not in skip:
                    ca = rt[:, 0, g, :].rearrange("p (q b) -> p q b", q=NTB)
                    sa = rt[:, 1, g, :].rearrange("p (q b) -> p q b", q=NTB)
                    u0 = xs[:, :, :, 0]
                    u1 = xs[:, :, :, 1]
                    tt = [
                        tpool.tile([P, NTB, GB], F32, tag=f"t{i}", name=f"t{i}_{g}")
                        for i in range(4)
                    ]
                    nc.vector.tensor_mul(out=tt[0][:], in0=u0, in1=ca)
                    nc.vector.tensor_mul(out=tt[1][:], in0=u1, in1=sa)
                    nc.vector.tensor_mul(out=tt[2][:], in0=u0, in1=sa)
                    nc.vector.tensor_mul(out=tt[3][:], in0=u1, in1=ca)
                    nc.vector.tensor_sub(out=u0, in0=tt[0][:], in1=tt[1][:])
                    nc.vector.tensor_add(out=u1, in0=tt[2][:], in1=tt[3][:])
                # channel dropout: zero dropped channels
                for c in (dropped if "r" not in skip else []):
                    nc.vector.tensor_scalar_mul(xs[:, :, :, c], xs[:, :, :, c], 0.0)
                if "m" in skip:
                    for tb in range(NTB):
                        nc.scalar.dma_start(
                            out=out[g, tb],
                            in_=xs[:, tb, :, :].bitcast(F32),
                        )
                    continue
                # time warp via banded matmul, per t-block
                if store4:
                    for tbq in range(NTB // 4):
                        ot = opool.tile([P, 4, F], F32, tag="ot4", name=f"ot4_{g}_{tbq}")
                        for k in range(4):
                            tb = tbq * 4 + k
                            ps = ppool.tile([P, F], F32, tag="ps", name=f"ps_{g}_{tb}")
                            n_mm = len(sched[tb])
                            for j, (sb, bi) in enumerate(sched[tb]):
                                nc.tensor.matmul(
                                    ps[:],
                                    wt[:, bi, :],
                                    xs[:, sb, :, :],
                                    start=(j == 0),
                                    stop=(j == n_mm - 1),
                                )
                            nc.scalar.copy(out=ot[:, k, :], in_=ps[:])
                        nc.scalar.dma_start(
                            out=out[g, tbq * 4:(tbq + 1) * 4].rearrange("q p f -> p q f"),
                            in_=ot[:],
                        )
                else:
                    for tb in range(NTB):
                        ps = ppool.tile([P, F], F32)
                        n_mm = len(sched[tb])
                        for j, (sb, bi) in enumerate(sched[tb]):
                            nc.tensor.matmul(
                                ps[:],
                                wt[:, bi, :],
                                xs[:, sb, :, :],
                                start=(j == 0),
                                stop=(j == n_mm - 1),
                            )
                        ot = opool.tile([P, F], F32)
                        nc.scalar.copy(out=ot[:], in_=ps[:])
                        nc.sync.dma_start(out=out[g, tb], in_=ot[:])
    nc.compile()
    return nc


def kernel(x, noise, scale_u, warp_noise, angle_u, chmask_u):
    import ml_dtypes

    x = np.ascontiguousarray(np.asarray(x, dtype=np.float32))
    noise = np.ascontiguousarray(np.asarray(noise, dtype=np.float32).astype(ml_dtypes.bfloat16))
    scale_u = np.asarray(scale_u, dtype=np.float32)
    warp_noise = np.asarray(warp_noise, dtype=np.float32)
    angle_u = np.asarray(angle_u, dtype=np.float32)
    chmask_u = np.asarray(chmask_u, dtype=np.float32)

    scale = np.float32(SCALE_LO) + scale_u[0] * np.float32(SCALE_HI - SCALE_LO)
    i0, frac = _warp_params(warp_noise)
    wmat, sched = _build_w_blocks(i0, frac, scale)
    nblk = wmat.shape[0]

    angle = (angle_u * np.float32(2.0 * np.pi) - np.float32(np.pi)).astype(np.float32)
    ca = np.cos(angle).astype(np.float32)
    sa = np.sin(angle).astype(np.float32)
    mask = np.asarray(chmask_u) > 0.1
    dropped = [c for c in range(C) if not mask[c]]
    rot_needed = bool(mask[0] or mask[1])

    # rotation constant tiles are built per core (angles differ per shard):
    # (2, NG, 128, 128); free col = tb*GB + b -> ca/sa of batch g*GB+b
    in_maps = []
    for core in range(N_CORES):
        b0 = core * BS
        ca_c = ca[b0:b0 + BS]
        sa_c = sa[b0:b0 + BS]
        rc = np.zeros((2, NG, P, P), np.float32)
        for g in range(NG):
            cols = np.tile(ca_c[g * GB:(g + 1) * GB], NTB)  # (128,) tb-major
            rc[0, g, :, :] = cols[None, :]
            cols = np.tile(sa_c[g * GB:(g + 1) * GB], NTB)
            rc[1, g, :, :] = cols[None, :]
        rc = rc.reshape(2 * NG, P, P)
        in_maps.append(
            {
                "x": x[b0:b0 + BS],
                "n": noise[b0:b0 + BS],
                "w": wmat,
                "rot": rc,
            }
        )

    iters = int(os.environ.get("KERNEL_ITERS", "1"))
    repeat = int(os.environ.get("KERNEL_REPEAT", "1"))
    nc = _build_nc(nblk, sched, dropped, rot_needed, iters=iters)

    res = run_bass_kernel_spmd(nc, in_maps, list(range(N_CORES)))
    if repeat > 1:
        import time as _time

        walls = []
        for _ in range(repeat):
            t0 = _time.perf_counter()
            res = run_bass_kernel_spmd(nc, in_maps, list(range(N_CORES)))
            walls.append(_time.perf_counter() - t0)
        print(
            f"KERNEL_WALLS iters={iters} min={min(walls)*1e3:.2f}ms "
            f"med={sorted(walls)[len(walls)//2]*1e3:.2f}ms all={[f'{w*1e3:.1f}' for w in walls]}",
            flush=True,
        )
    outs = []
    for i in range(N_CORES):
        o = res.results[i]["out"].reshape(NG, NTB, P, GB, C)
        # (g, tb, p, b, c) -> (g, b, tb, p, c) -> (BS, T, C)
        outs.append(np.ascontiguousarray(o.transpose(0, 3, 1, 2, 4)).reshape(BS, T, C))
    return np.concatenate(outs, axis=0)
